# revision 1
# baseline (speedup 1.0000x reference)
"""Trainium2 Bass kernel for nn_CFAConv (cross-feature attention + conv block).

Self-contained: takes full unsharded inputs, shards (batch, image-half) across
8 NeuronCores, runs one SPMD Bass/Tile NEFF, and combines partial results on
the host.

Math (validated against the jax reference in numpy):
  x1 = w_q@in0 + b_q ; x2 = w_k@in0 + b_k ; x3 = w_v@in1 + b_v  (1x1 convs)
  aff = softmax_j(x2^T x3) ; x0 = x1 @ aff
  x0' = gelu(bn0(w_o@x0 + b_o))
  y = gelu(bn(conv3x3(concat(x0', in0)))) ; y = gelu(bn(conv3x3(y)))
  out = max_spatial(y + x0')
On-device simplifications:
  - softmax over j is invariant to per-column shifts => b_k drops entirely
  - x2^T(x3 + b_v) = x2^T x3 + (x2^T b_v) 1^T    => fold b_v into x3
  - (x1 + b_q 1^T) @ aff = x1@aff + b_q 1^T (aff columns sum to 1)
    => fold w_o@b_q into the out-projection bias (host-side)
  - eval-mode BN folds to per-channel scale/bias, fused into the gelu ACT op
  - softmax normalization deferred past the x1@exp(S) matmul (divide x0 by
    column sums), sums via a ones-row matmul
  - no max-subtraction in softmax: |S| <= ~60 here; exp fits fp32 (max ~e88)
Precision: bf16 operands everywhere with fp32 PSUM accumulation (validated
3.2e-3 final rel err in a numpy bit-model vs the 2e-2 budget); softmax sums
and normalization in fp32.
Sharding: 8 cores = (4 batches) x (top/bottom image half). Each core computes
a 34-row window (32 owned + halo) so the two 3x3 convs need no communication;
per-row maxes [256, 34] go to the host which slices owned rows and reduces.
"""

from contextlib import ExitStack

import ml_dtypes
import numpy as np

import concourse.bass as bass
import concourse.tile as tile
from concourse import bacc, mybir
from concourse.bass_utils import run_bass_kernel_spmd

B, C, H, W = 4, 256, 64, 64
Ch = C // 2          # 128
N = H * W            # 4096
ROWS = 34            # per-core row window (32 owned + 2 halo)
KW = ROWS * W        # 2176 window positions
EPS = 1e-5

F32 = mybir.dt.float32
BF16 = mybir.dt.bfloat16
AF = mybir.ActivationFunctionType
AX = mybir.AxisListType
BF16NP = ml_dtypes.bfloat16

# attention k-tiles over the 2176-column window
K_TILES = [(0, 512), (512, 512), (1024, 512), (1536, 512), (2048, 128)]
# conv output row-tiles (local rows 1..34 of the 36-row padded buffer)
ROW_TILES = [(1, 8), (9, 8), (17, 8), (25, 8), (33, 2)]

_CACHED = {}


def build_program():
    nc = bacc.Bacc("TRN2", target_bir_lowering=False, debug=False)

    def din(name, shape, dt=F32):
        return nc.dram_tensor(name, shape, dt, kind="ExternalInput").ap()

    in0b_d = din("in0b", [C, N], BF16)
    in0w = din("in0w", [C, KW], BF16)
    in1b_d = din("in1b", [C, KW], BF16)
    wq_t = din("wq_t", [C, Ch], BF16)     # (c, i)
    wk_t = din("wk_t", [C, Ch], BF16)
    wv_t = din("wv_t", [C, Ch], BF16)
    wo_t = din("wo_t", [Ch, C], BF16)     # (i, o)
    bv = din("bv", [Ch, 1])
    ao = din("ao", [C, 1])
    bo = din("bo", [C, 1])
    w0t = din("w0t", [9, 2 * C, C], BF16)  # (tap, ci, o)
    a0 = din("a0", [C, 1])
    b0 = din("b0", [C, 1])
    w1t = din("w1t", [9, C, C], BF16)
    a1 = din("a1", [C, 1])
    b1 = din("b1", [C, 1])
    out = nc.dram_tensor("out", [C, ROWS], F32, kind="ExternalOutput").ap()

    with tile.TileContext(nc) as tc, ExitStack() as ctx:
        persist = ctx.enter_context(tc.tile_pool(name="persist", bufs=1))
        psum = ctx.enter_context(tc.tile_pool(name="psum", bufs=2, space="PSUM"))
        psum1 = ctx.enter_context(tc.tile_pool(name="psum1", bufs=2, space="PSUM"))
        small = ctx.enter_context(tc.tile_pool(name="small", bufs=3))

        # ---- inputs: bf16 quarters of in0 (one DMA each: per-slice deps
        # because Tile dependencies are whole-tile) ----
        in0q = [persist.tile([128, 2, 512], BF16, tag=f"in0q{q}",
                             name=f"in0q{q}") for q in range(8)]
        nc.sync.dma_start(
            out=in0q[0],
            in_=in0b_d.rearrange("(a p) n -> p a n", a=2)[:, :, 0:512])
        # weights needed first
        wk_s = persist.tile([128, 2, Ch], BF16, tag="wk")
        nc.sync.dma_start(out=wk_s, in_=wk_t.rearrange("(a p) n -> p a n", a=2))
        wq_s = persist.tile([128, 2, Ch], BF16, tag="wq")
        nc.sync.dma_start(out=wq_s, in_=wq_t.rearrange("(a p) n -> p a n", a=2))
        wv_s = persist.tile([128, 2, Ch], BF16, tag="wv")
        nc.sync.dma_start(out=wv_s, in_=wv_t.rearrange("(a p) n -> p a n", a=2))
        for q in range(1, 8):
            nc.sync.dma_start(
                out=in0q[q],
                in_=in0b_d.rearrange("(a p) n -> p a n", a=2)[
                    :, :, q * 512:(q + 1) * 512])
        in1q = [persist.tile([128, 2, 1152], BF16, tag=f"in1q{q}",
                             name=f"in1q{q}") for q in range(2)]
        nc.sync.dma_start(
            out=in1q[0][:, :, :1024],
            in_=in1b_d.rearrange("(a p) n -> p a n", a=2)[:, :, 0:1024])
        nc.sync.dma_start(
            out=in1q[1],
            in_=in1b_d.rearrange("(a p) n -> p a n", a=2)[:, :, 1024:KW])
        wo_s = persist.tile([128, C], BF16, tag="wo")
        nc.sync.dma_start(out=wo_s, in_=wo_t)
        bv_s = persist.tile([128, 1], F32, tag="bv")
        nc.sync.dma_start(out=bv_s, in_=bv)
        ao_s = persist.tile([128, 2], F32, tag="ao")
        bo_s = persist.tile([128, 2], F32, tag="bo")
        a0_s = persist.tile([128, 2], F32, tag="a0")
        b0_s = persist.tile([128, 2], F32, tag="b0")
        a1_s = persist.tile([128, 2], F32, tag="a1")
        b1_s = persist.tile([128, 2], F32, tag="b1")
        for t_s, t_d in ((ao_s, ao), (bo_s, bo), (a0_s, a0), (b0_s, b0),
                         (a1_s, a1), (b1_s, b1)):
            nc.sync.dma_start(out=t_s,
                              in_=t_d.rearrange("(a p) o -> p (a o)", a=2))
        ones_s = persist.tile([128, 1], BF16, tag="ones")
        nc.vector.memset(ones_s, 1.0)

        # ---- projections: x2 [ch, N], x1T [j, i], x3 [ch, KW] (all bf16) --
        x2_s = persist.tile([128, N], BF16, tag="x2")
        x1t_s = persist.tile([128, 32, Ch], BF16, tag="x1t")
        x3_s = persist.tile([128, KW], BF16, tag="x3")

        for jc in range(8):
            q, rr = jc, 0
            ps2 = psum.tile([128, 2, 512], F32, tag="ps_S")
            for cc in range(2):
                nc.tensor.matmul(ps2[:, 0, :], wk_s[:, cc, :],
                                 in0q[q][:, cc, rr:rr + 512],
                                 start=(cc == 0), stop=(cc == 1))
            nc.vector.tensor_copy(x2_s[:, jc * 512:(jc + 1) * 512], ps2[:, 0, :])
            for js in range(4):
                ps1 = psum1.tile([128, 512], F32, tag="ps_acc")
                for cc in range(2):
                    nc.tensor.matmul(
                        ps1[:, :Ch],
                        in0q[q][:, cc, rr + js * 128:rr + (js + 1) * 128],
                        wq_s[:, cc, :],
                        start=(cc == 0), stop=(cc == 1))
                nc.vector.tensor_copy(x1t_s[:, jc * 4 + js, :], ps1[:, :Ch])

        for k0, ksz in K_TILES:
            iq, off = (0, k0) if k0 < 1024 else (1, k0 - 1024)
            ps3 = psum.tile([128, 2, 512], F32, tag="ps_S")
            for cc in range(2):
                nc.tensor.matmul(ps3[:, 0, :ksz], wv_s[:, cc, :],
                                 in1q[iq][:, cc, off:off + ksz],
                                 start=(cc == 0), stop=(cc == 1))
            # x3 = psum + b_v : folds the v-bias into the affinity logits
            nc.vector.tensor_scalar_add(x3_s[:, k0:k0 + ksz], ps3[:, 0, :ksz],
                                        bv_s)

        # ---- conv buffers (x0' evictions write into ybuf) ----
        convbuf = ctx.enter_context(tc.tile_pool(name="convbuf", bufs=1))
        ybuf = [convbuf.tile([128, 36, 66], BF16, tag=f"y{ci}", name=f"ybuf{ci}")
                for ci in range(4)]
        c0buf = [convbuf.tile([128, 36, 66], BF16, tag=f"c0{oc}", name=f"c0buf{oc}")
                 for oc in range(2)]
        for tl in ybuf + c0buf:
            # zero the pad ring (write-only memset; reading uninitialized
            # SBUF can produce NaNs)
            nc.vector.memset(tl[:, 0, :], 0.0)
            nc.vector.memset(tl[:, 35, :], 0.0)
            nc.vector.memset(tl[:, 1:35, 0:1], 0.0)
            nc.vector.memset(tl[:, 1:35, 65:66], 0.0)
        # in0 window DMA (consumed into ybuf after attention)
        in0w_raw = persist.tile([128, 2, ROWS, W], BF16, tag="in0w_raw")
        nc.sync.dma_start(
            out=in0w_raw,
            in_=in0w.rearrange("(a p) (r w) -> p a r w", a=2, w=W))

        # ---- attention: S = x2^T x3, exp, x0 = x1 @ exp, sums, normalize ----
        attn = ctx.enter_context(tc.tile_pool(name="attn", bufs=4))
        dram = ctx.enter_context(tc.tile_pool(name="dram", bufs=5, space="DRAM"))
        x0n_s = persist.tile([128, KW], BF16, tag="x0n")
        for k0, ksz in K_TILES:
            # four quarter-tiles under one bufs=4 tag: stage-2 consumes a
            # quarter while later quarters' exps still run, and the next
            # k-tile's exps begin as soon as a quarter is drained
            expS_h = [attn.tile([128, 8, 512], BF16, tag="expS",
                                name=f"expS{k0}_{h}") for h in range(4)]
            for mh in range(16):  # chunk pairs
                sp = psum.tile([128, 2, 512], F32, tag="ps_S")
                for i in range(2):
                    m = 2 * mh + i
                    nc.tensor.matmul(
                        sp[:, i, :ksz],
                        x2_s[:, m * 128:(m + 1) * 128],
                        x3_s[:, k0:k0 + ksz],
                        start=True, stop=True)
                eh = expS_h[mh // 4]
                nc.scalar.activation(
                    eh[:, (2 * mh) % 8:(2 * mh) % 8 + 2, :ksz],
                    sp[:, :, :ksz], AF.Exp)
            # bf16 pair+quad pre-sums on DVE shrink the ones-matmul to 8
            # chunks (s error ~0.2%, only scales the softmax normalization)
            quads = []
            for h in range(4):
                pair = attn.tile([128, 4, 512], BF16, tag="pair",
                                 name=f"pair{k0}_{h}")
                for i in range(4):
                    nc.vector.tensor_add(pair[:, i, :ksz],
                                         expS_h[h][:, 2 * i, :ksz],
                                         expS_h[h][:, 2 * i + 1, :ksz])
                quad = attn.tile([128, 2, 512], BF16, tag="quad",
                                 name=f"quad{k0}_{h}")
                for i in range(2):
                    nc.vector.tensor_add(quad[:, i, :ksz],
                                         pair[:, 2 * i, :ksz],
                                         pair[:, 2 * i + 1, :ksz])
                quads.append(quad)
            x0p = psum1.tile([128, 512], F32, tag="ps_acc")
            ssum = psum1.tile([1, 512], F32, tag="ps_sum")
            for m in range(32):
                eSm = expS_h[m // 8][:, m % 8, :ksz]
                nc.tensor.matmul(x0p[:, :ksz], x1t_s[:, m, :], eSm,
                                 start=(m == 0), stop=(m == 31))
            for qq in range(8):
                nc.tensor.matmul(ssum[:, :ksz], ones_s,
                                 quads[qq // 2][:, qq % 2, :ksz],
                                 start=(qq == 0), stop=(qq == 7))
            sinv = small.tile([1, 512], F32, tag="sinv")
            nc.vector.reciprocal(sinv[:, :ksz], ssum[:, :ksz])
            sinv_d = dram.tile([1, 512], F32, tag="sinv_d")
            nc.sync.dma_start(out=sinv_d[:, :ksz], in_=sinv[:, :ksz])
            sinvb = small.tile([128, 512], F32, tag="sinvb")
            nc.sync.dma_start(
                out=sinvb[:, :ksz],
                in_=sinv_d[:, :ksz].partition_broadcast(128)[:, 0, :])
            nc.vector.tensor_mul(x0n_s[:, k0:k0 + ksz], x0p[:, :ksz],
                                 sinvb[:, :ksz])

        # in0 window -> ybuf channels 256..511 (DVE; ACT is busy with exps)
        for ci in range(2):
            nc.vector.tensor_copy(ybuf[2 + ci][:, 1:35, 1:65], in0w_raw[:, ci])

        # ---- conv0 weights (loaded during attention) ----
        w0_s = persist.tile([128, 36, C], BF16, tag="w0")
        for t9 in range(9):
            nc.sync.dma_start(
                out=w0_s[:, t9 * 4:(t9 + 1) * 4, :],
                in_=w0t[t9].rearrange("(a p) o -> p a o", a=4))

        # ---- out-projection + bn0 + gelu -> x0' into ybuf channels 0..255 --
        for kt, (k0, ksz) in enumerate(K_TILES):
            nr = ksz // W  # rows in this k-tile
            for oc in range(2):
                po = psum.tile([128, 2, 512], F32, tag="ps_S")
                nc.tensor.matmul(po[:, 0, :ksz],
                                 wo_s[:, oc * 128:(oc + 1) * 128],
                                 x0n_s[:, k0:k0 + ksz],
                                 start=True, stop=True)
                nc.scalar.activation(
                    ybuf[oc][:, 1 + kt * 8:1 + kt * 8 + nr, 1:65],
                    po[:, 0, :ksz].rearrange("p (r w) -> p r w", w=W),
                    AF.Gelu, bias=bo_s[:, oc:oc + 1], scale=ao_s[:, oc:oc + 1])

        # ---- conv1 weights (loaded during conv0) ----
        w1_s = persist.tile([128, 18, C], BF16, tag="w1")
        for t9 in range(9):
            nc.sync.dma_start(
                out=w1_s[:, t9 * 2:(t9 + 1) * 2, :],
                in_=w1t[t9].rearrange("(a p) o -> p a o", a=2))

        # ---- conv0: 512 -> 256, 3x3, bn + gelu ----
        for r0, nr in ROW_TILES:
            for oc in range(2):
                pc = psum.tile([128, 2, 512], F32, tag="ps_S")
                pcv = pc[:, 0, :nr * W].rearrange("p (r w) -> p r w", w=W)
                i_mm = 0
                for t9 in range(9):
                    dh, dw = divmod(t9, 3)
                    for ci in range(4):
                        nc.tensor.matmul(
                            pcv,
                            w0_s[:, t9 * 4 + ci, oc * 128:(oc + 1) * 128],
                            ybuf[ci][:, r0 + dh - 1:r0 + dh - 1 + nr, dw:dw + W],
                            start=(i_mm == 0), stop=(i_mm == 35))
                        i_mm += 1
                nc.scalar.activation(
                    c0buf[oc][:, r0:r0 + nr, 1:65], pcv,
                    AF.Gelu, bias=b0_s[:, oc:oc + 1], scale=a0_s[:, oc:oc + 1])

        # ---- conv1: 256 -> 256, 3x3, bn + gelu, + x0' residual, row-max ----
        outs = [persist.tile([128, ROWS], F32, tag=f"out{oc}", name=f"outs{oc}")
                for oc in range(2)]
        for r0, nr in ROW_TILES:
            for oc in range(2):
                pc = psum.tile([128, 2, 512], F32, tag="ps_S")
                pcv = pc[:, 0, :nr * W].rearrange("p (r w) -> p r w", w=W)
                i_mm = 0
                for t9 in range(9):
                    dh, dw = divmod(t9, 3)
                    for ci in range(2):
                        nc.tensor.matmul(
                            pcv,
                            w1_s[:, t9 * 2 + ci, oc * 128:(oc + 1) * 128],
                            c0buf[ci][:, r0 + dh - 1:r0 + dh - 1 + nr, dw:dw + W],
                            start=(i_mm == 0), stop=(i_mm == 17))
                        i_mm += 1
                tmp = small.tile([128, 512], F32, tag="scratch")
                nc.scalar.activation(tmp[:, :nr * W], pc[:, 0, :nr * W], AF.Gelu,
                                     bias=b1_s[:, oc:oc + 1],
                                     scale=a1_s[:, oc:oc + 1])
                res = small.tile([128, 512], F32, tag="scratch")
                nc.vector.tensor_add(
                    res[:, :nr * W].rearrange("p (r w) -> p r w", w=W),
                    tmp[:, :nr * W].rearrange("p (r w) -> p r w", w=W),
                    ybuf[oc][:, r0:r0 + nr, 1:65])
                nc.vector.reduce_max(
                    outs[oc][:, r0 - 1:r0 - 1 + nr],
                    res[:, :nr * W].rearrange("p (r w) -> p r w", w=W),
                    axis=AX.X)
        for oc in range(2):
            nc.sync.dma_start(out=out[oc * 128:(oc + 1) * 128, :], in_=outs[oc])

    nc.compile()
    return nc


def _prep_maps(inputs):
    """Host-side input prep: slicing, transposes, BN folding, bf16 casts."""
    f = np.float32
    in0 = np.ascontiguousarray(np.asarray(inputs["inputs_0"], f).reshape(B, C, N))
    in1 = np.ascontiguousarray(np.asarray(inputs["inputs_1"], f).reshape(B, C, N))
    g = {k: np.asarray(v, f) for k, v in inputs.items()}

    def fold(gm, bt, m, v, conv_b):
        a = (gm / np.sqrt(v + EPS)).astype(f)
        return a, (bt - m * a + a * conv_b).astype(f)

    a_bn, b_bn = fold(g["bn0_g"], g["bn0_b"], g["bn0_m"], g["bn0_v"],
                      g["b_o"] + g["w_o"] @ g["b_q"])
    a0, b0 = fold(g["cb_bn0_g"], g["cb_bn0_b"], g["cb_bn0_m"], g["cb_bn0_v"],
                  g["cb_b0"])
    a1, b1 = fold(g["cb_bn1_g"], g["cb_bn1_b"], g["cb_bn1_m"], g["cb_bn1_v"],
                  g["cb_b1"])

    shared = {
        "wq_t": np.ascontiguousarray(g["w_q"].T).astype(BF16NP),
        "wk_t": np.ascontiguousarray(g["w_k"].T).astype(BF16NP),
        "wv_t": np.ascontiguousarray(g["w_v"].T).astype(BF16NP),
        "wo_t": np.ascontiguousarray(g["w_o"].T).astype(BF16NP),
        "bv": np.ascontiguousarray(g["b_v"].reshape(Ch, 1)),
        "ao": a_bn.reshape(C, 1), "bo": b_bn.reshape(C, 1),
        "w0t": np.ascontiguousarray(
            g["cb_w0"].transpose(2, 3, 1, 0).reshape(9, 2 * C, C)).astype(BF16NP),
        "a0": a0.reshape(C, 1), "b0": b0.reshape(C, 1),
        "w1t": np.ascontiguousarray(
            g["cb_w1"].transpose(2, 3, 1, 0).reshape(9, C, C)).astype(BF16NP),
        "a1": a1.reshape(C, 1), "b1": b1.reshape(C, 1),
    }
    maps = []
    for b in range(B):
        in0b16 = in0[b].astype(BF16NP)
        for half in range(2):
            w0r = 0 if half == 0 else 30
            sl = slice(w0r * W, (w0r + ROWS) * W)
            maps.append({
                "in0b": in0b16,
                "in0w": np.ascontiguousarray(in0b16[:, sl]),
                "in1b": np.ascontiguousarray(in1[b][:, sl]).astype(BF16NP),
                **shared,
            })
    return maps


def kernel(**inputs):
    if "nc" not in _CACHED:
        _CACHED["nc"] = build_program()
    nc = _CACHED["nc"]
    maps = _prep_maps(inputs)
    res = run_bass_kernel_spmd(nc, maps, core_ids=list(range(8)))
    out = np.zeros((B, C), np.float32)
    for b in range(B):
        top = res.results[2 * b]["out"][:, 0:32].max(axis=1)
        bot = res.results[2 * b + 1]["out"][:, 2:34].max(axis=1)
        out[b] = np.maximum(top, bot)
    return out



# revision 6
# speedup vs baseline: 1.2471x; 1.2471x over previous
"""Trainium2 Bass kernel for nn_CFAConv (cross-feature attention + conv block).

Self-contained: takes full unsharded inputs, shards (batch, image-half) across
8 NeuronCores, runs one SPMD Bass/Tile NEFF, and combines partial results on
the host.

Math (validated against the jax reference in numpy):
  x1 = w_q@in0 + b_q ; x2 = w_k@in0 + b_k ; x3 = w_v@in1 + b_v  (1x1 convs)
  aff = softmax_j(x2^T x3) ; x0 = x1 @ aff
  x0' = gelu(bn0(w_o@x0 + b_o))
  y = gelu(bn(conv3x3(concat(x0', in0)))) ; y = gelu(bn(conv3x3(y)))
  out = max_spatial(y + x0')
On-device simplifications:
  - softmax over j is invariant to per-column shifts => b_k drops entirely
  - x2^T(x3 + b_v) = x2^T x3 + (x2^T b_v) 1^T    => fold b_v into x3
  - (x1 + b_q 1^T) @ aff = x1@aff + b_q 1^T (aff columns sum to 1)
    => fold w_o@b_q into the out-projection bias (host-side)
  - eval-mode BN folds to per-channel scale/bias, fused into the gelu ACT op
  - softmax normalization deferred past the x1@exp(S) matmul (divide x0 by
    column sums); sums via a 5-level bf16 DVE pre-sum tree + one ones-matmul
  - no max-subtraction in softmax: |S| <= ~60 here; exp fits fp32 (max ~e88)
Precision: bf16 operands with fp32 PSUM accumulation for the attention path;
the two 3x3 convs run in fp8e4m3 with DoubleRow perf mode (2 contraction
tiles per pass at 0.5 cycles/row):
  - conv0 x0'-half: weights + acts naive fp8 (x0' is small vs in0 => cheap)
  - conv0 in0-half: weights hi+lo fp8 split, in0 hi+lo fp8 split (host-side),
    3-term product (Wh Xh + Wh Xl + Wl Xh)
  - conv1: weights hi+lo (host), c0 hi+lo split on DVE, 3-term
  (numpy bit-model: 1.3e-2 final rel err vs the 2e-2 budget)
Sharding: 8 cores = (4 batches) x (top/bottom image half). Each core computes
a 34-row window (32 owned + halo) so the two 3x3 convs need no communication;
per-row maxes [256, 34] go to the host which slices owned rows and reduces.
"""

from contextlib import ExitStack

import ml_dtypes
import numpy as np

import concourse.bass as bass
import concourse.tile as tile
from concourse import bacc, mybir
from concourse.bass_utils import run_bass_kernel_spmd

B, C, H, W = 4, 256, 64, 64
Ch = C // 2          # 128
N = H * W            # 4096
ROWS = 34            # per-core row window (32 owned + 2 halo)
KW = ROWS * W        # 2176 window positions
EPS = 1e-5

F32 = mybir.dt.float32
BF16 = mybir.dt.bfloat16
F8 = mybir.dt.float8e4
AF = mybir.ActivationFunctionType
AX = mybir.AxisListType
DR = mybir.MatmulPerfMode.DoubleRow
BF16NP = ml_dtypes.bfloat16
F8NP = ml_dtypes.float8_e4m3

# attention k-tiles over the 2176-column window
K_TILES = [(0, 512), (512, 512), (1024, 512), (1536, 512), (2048, 128)]
# conv output row-tiles (local rows 1..34 of the 36-row padded buffer)
ROW_TILES = [(1, 8), (9, 8), (17, 8), (25, 8), (33, 2)]

_CACHED = {}


def build_program():
    nc = bacc.Bacc("TRN2", target_bir_lowering=False, debug=False)

    def din(name, shape, dt=F32):
        return nc.dram_tensor(name, shape, dt, kind="ExternalInput").ap()

    in0b_d = din("in0b", [C, N], BF16)
    in1b_d = din("in1b", [C, KW], BF16)
    # in0 conv window, fp8 hi/lo, pre-padded to 66 cols (zero side columns)
    in0h_d = din("in0h", [C, ROWS * 66], F8)
    in0l_d = din("in0l", [C, ROWS * 66], F8)
    wq_t = din("wq_t", [C, Ch], BF16)     # (c, i)
    wk_t = din("wk_t", [C, Ch], BF16)
    wv_t = din("wv_t", [C, Ch], BF16)
    wo_t = din("wo_t", [Ch, C], BF16)     # (i, o)
    bv = din("bv", [Ch, 1])
    ao = din("ao", [C, 1])
    bo = din("bo", [C, 1])
    w0x_d = din("w0x", [9, C, C], F8)     # (tap, ci[x0-half], o) naive fp8
    w0inh_d = din("w0inh", [9, C, C], F8)  # (tap, ci[in0-half], o) hi
    w0inl_d = din("w0inl", [9, C, C], F8)  # lo
    a0 = din("a0", [C, 1])
    b0 = din("b0", [C, 1])
    w1h_d = din("w1h", [9, C, C], F8)
    w1l_d = din("w1l", [9, C, C], F8)
    a1 = din("a1", [C, 1])
    b1 = din("b1", [C, 1])
    out = nc.dram_tensor("out", [C, ROWS], F32, kind="ExternalOutput").ap()

    with tile.TileContext(nc) as tc, ExitStack() as ctx:
        persist = ctx.enter_context(tc.tile_pool(name="persist", bufs=1))
        psum = ctx.enter_context(tc.tile_pool(name="psum", bufs=2, space="PSUM"))
        psum1 = ctx.enter_context(tc.tile_pool(name="psum1", bufs=2, space="PSUM"))
        small = ctx.enter_context(tc.tile_pool(name="small", bufs=3))

        # ---- inputs: bf16 quarters of in0 (one DMA each: per-slice deps
        # because Tile dependencies are whole-tile) ----
        in0q = [persist.tile([128, 2, 512], BF16, tag=f"in0q{q}",
                             name=f"in0q{q}") for q in range(8)]
        nc.sync.dma_start(
            out=in0q[0],
            in_=in0b_d.rearrange("(a p) n -> p a n", a=2)[:, :, 0:512])
        # weights needed first
        wk_s = persist.tile([128, 2, Ch], BF16, tag="wk")
        nc.sync.dma_start(out=wk_s, in_=wk_t.rearrange("(a p) n -> p a n", a=2))
        wq_s = persist.tile([128, 2, Ch], BF16, tag="wq")
        nc.sync.dma_start(out=wq_s, in_=wq_t.rearrange("(a p) n -> p a n", a=2))
        wv_s = persist.tile([128, 2, Ch], BF16, tag="wv")
        nc.sync.dma_start(out=wv_s, in_=wv_t.rearrange("(a p) n -> p a n", a=2))
        for q in range(1, 8):
            nc.sync.dma_start(
                out=in0q[q],
                in_=in0b_d.rearrange("(a p) n -> p a n", a=2)[
                    :, :, q * 512:(q + 1) * 512])
        in1q = [persist.tile([128, 2, 1152], BF16, tag=f"in1q{q}",
                             name=f"in1q{q}") for q in range(2)]
        nc.sync.dma_start(
            out=in1q[0][:, :, :1024],
            in_=in1b_d.rearrange("(a p) n -> p a n", a=2)[:, :, 0:1024])
        nc.sync.dma_start(
            out=in1q[1],
            in_=in1b_d.rearrange("(a p) n -> p a n", a=2)[:, :, 1024:KW])
        wo_s = persist.tile([128, C], BF16, tag="wo")
        nc.sync.dma_start(out=wo_s, in_=wo_t)
        bv_s = persist.tile([128, 1], F32, tag="bv")
        nc.sync.dma_start(out=bv_s, in_=bv)
        ao_s = persist.tile([128, 2], F32, tag="ao")
        bo_s = persist.tile([128, 2], F32, tag="bo")
        a0_s = persist.tile([128, 2], F32, tag="a0")
        b0_s = persist.tile([128, 2], F32, tag="b0")
        a1_s = persist.tile([128, 2], F32, tag="a1")
        b1_s = persist.tile([128, 2], F32, tag="b1")
        for t_s, t_d in ((ao_s, ao), (bo_s, bo), (a0_s, a0), (b0_s, b0),
                         (a1_s, a1), (b1_s, b1)):
            nc.sync.dma_start(out=t_s,
                              in_=t_d.rearrange("(a p) o -> p (a o)", a=2))
        ones_s = persist.tile([128, 1], BF16, tag="ones")
        nc.vector.memset(ones_s, 1.0)

        # ---- projections: x2 [ch, N], x1T [j, i], x3 [ch, KW] (all bf16) --
        x2_s = persist.tile([128, N], BF16, tag="x2")
        x1t_s = persist.tile([128, 32, Ch], BF16, tag="x1t")
        x3_s = persist.tile([128, KW], BF16, tag="x3")

        for jc in range(8):
            q, rr = jc, 0
            ps2 = psum.tile([128, 2, 512], F32, tag="ps_S")
            for cc in range(2):
                nc.tensor.matmul(ps2[:, 0, :], wk_s[:, cc, :],
                                 in0q[q][:, cc, rr:rr + 512],
                                 start=(cc == 0), stop=(cc == 1))
            nc.vector.tensor_copy(x2_s[:, jc * 512:(jc + 1) * 512], ps2[:, 0, :])
            for js in range(4):
                ps1 = psum1.tile([128, 512], F32, tag="ps_acc")
                for cc in range(2):
                    nc.tensor.matmul(
                        ps1[:, :Ch],
                        in0q[q][:, cc, rr + js * 128:rr + (js + 1) * 128],
                        wq_s[:, cc, :],
                        start=(cc == 0), stop=(cc == 1))
                nc.vector.tensor_copy(x1t_s[:, jc * 4 + js, :], ps1[:, :Ch])

        for k0, ksz in K_TILES:
            iq, off = (0, k0) if k0 < 1024 else (1, k0 - 1024)
            ps3 = psum.tile([128, 2, 512], F32, tag="ps_S")
            for cc in range(2):
                nc.tensor.matmul(ps3[:, 0, :ksz], wv_s[:, cc, :],
                                 in1q[iq][:, cc, off:off + ksz],
                                 start=(cc == 0), stop=(cc == 1))
            # x3 = psum + b_v : folds the v-bias into the affinity logits
            nc.vector.tensor_scalar_add(x3_s[:, k0:k0 + ksz], ps3[:, 0, :ksz],
                                        bv_s)

        # ---- conv buffers (fp8, padded 36x66 with zero ring) ----
        convbuf = ctx.enter_context(tc.tile_pool(name="convbuf", bufs=1))
        ybuf = convbuf.tile([128, 2, 36, 66], F8, tag="ybuf")   # x0' chunks
        in0h_s = convbuf.tile([128, 2, 36, 66], F8, tag="in0h")
        in0l_s = convbuf.tile([128, 2, 36, 66], F8, tag="in0l")
        c0h = convbuf.tile([128, 2, 36, 66], F8, tag="c0h")
        c0l = convbuf.tile([128, 2, 36, 66], F8, tag="c0l")
        c0f = convbuf.tile([128, 2, ROWS, W], BF16, tag="c0f")
        for tl in (ybuf, in0h_s, in0l_s, c0h, c0l):
            # zero the pad ring (write-only memset; reading uninitialized
            # SBUF can produce NaNs)
            nc.vector.memset(tl[:, :, 0, :], 0.0)
            nc.vector.memset(tl[:, :, 35, :], 0.0)
            if tl is in0h_s or tl is in0l_s:
                continue  # side columns arrive zero-padded via the DMA
            nc.vector.memset(tl[:, :, 1:35, 0:1], 0.0)
            nc.vector.memset(tl[:, :, 1:35, 65:66], 0.0)
        # in0 conv window ships as fp8 hi/lo straight into the padded tiles
        # (host pre-pads the 66-col side ring so the DMA stays 3-dim)
        nc.sync.dma_start(
            out=in0h_s[:, :, 1:35, :],
            in_=in0h_d.rearrange("(a p) n -> p a n", a=2))
        nc.sync.dma_start(
            out=in0l_s[:, :, 1:35, :],
            in_=in0l_d.rearrange("(a p) n -> p a n", a=2))

        # ---- attention: S = x2^T x3, exp, x0 = x1 @ exp, sums, normalize ----
        attn = ctx.enter_context(tc.tile_pool(name="attn", bufs=4))
        attn2 = ctx.enter_context(tc.tile_pool(name="attn2", bufs=2))
        dram = ctx.enter_context(tc.tile_pool(name="dram", bufs=5, space="DRAM"))
        x0n_s = persist.tile([128, KW], BF16, tag="x0n")
        for k0, ksz in K_TILES:
            # four quarter-tiles under one bufs=4 tag: stage-2 consumes a
            # quarter while later quarters' exps still run, and the next
            # k-tile's exps begin as soon as a quarter is drained
            expS_h = [attn.tile([128, 8, 512], BF16, tag="expS",
                                name=f"expS{k0}_{h}") for h in range(4)]
            for mh in range(16):  # chunk pairs
                sp = psum.tile([128, 2, 512], F32, tag="ps_S")
                for i in range(2):
                    m = 2 * mh + i
                    nc.tensor.matmul(
                        sp[:, i, :ksz],
                        x2_s[:, m * 128:(m + 1) * 128],
                        x3_s[:, k0:k0 + ksz],
                        start=True, stop=True)
                eh = expS_h[mh // 4]
                nc.scalar.activation(
                    eh[:, (2 * mh) % 8:(2 * mh) % 8 + 2, :ksz],
                    sp[:, :, :ksz], AF.Exp)
            # 5-level bf16 pre-sum tree on DVE collapses the softmax
            # column-sum to ONE ones-matmul pass (sum error ~0.3%, only
            # scales the normalization)
            octs = attn2.tile([128, 4, 512], BF16, tag="oct")
            for h in range(4):
                pair = attn.tile([128, 4, 512], BF16, tag="pair",
                                 name=f"pair{k0}_{h}")
                for i in range(4):
                    nc.vector.tensor_add(pair[:, i, :ksz],
                                         expS_h[h][:, 2 * i, :ksz],
                                         expS_h[h][:, 2 * i + 1, :ksz])
                quad = attn.tile([128, 2, 512], BF16, tag="quad",
                                 name=f"quad{k0}_{h}")
                for i in range(2):
                    nc.vector.tensor_add(quad[:, i, :ksz],
                                         pair[:, 2 * i, :ksz],
                                         pair[:, 2 * i + 1, :ksz])
                nc.vector.tensor_add(octs[:, h, :ksz], quad[:, 0, :ksz],
                                     quad[:, 1, :ksz])
            hexs = attn2.tile([128, 2, 512], BF16, tag="hex")
            for i in range(2):
                nc.vector.tensor_add(hexs[:, i, :ksz], octs[:, 2 * i, :ksz],
                                     octs[:, 2 * i + 1, :ksz])
            top = attn2.tile([128, 512], BF16, tag="top")
            nc.vector.tensor_add(top[:, :ksz], hexs[:, 0, :ksz],
                                 hexs[:, 1, :ksz])
            x0p = psum1.tile([128, 512], F32, tag="ps_acc")
            ssum = psum1.tile([1, 512], F32, tag="ps_sum")
            for m in range(32):
                eSm = expS_h[m // 8][:, m % 8, :ksz]
                nc.tensor.matmul(x0p[:, :ksz], x1t_s[:, m, :], eSm,
                                 start=(m == 0), stop=(m == 31))
            nc.tensor.matmul(ssum[:, :ksz], ones_s, top[:, :ksz],
                             start=True, stop=True)
            sinv = small.tile([1, 512], F32, tag="sinv")
            nc.vector.reciprocal(sinv[:, :ksz], ssum[:, :ksz])
            sinv_d = dram.tile([1, 512], F32, tag="sinv_d")
            nc.sync.dma_start(out=sinv_d[:, :ksz], in_=sinv[:, :ksz])
            sinvb = small.tile([128, 512], F32, tag="sinvb")
            nc.sync.dma_start(
                out=sinvb[:, :ksz],
                in_=sinv_d[:, :ksz].partition_broadcast(128)[:, 0, :])
            nc.vector.tensor_mul(x0n_s[:, k0:k0 + ksz], x0p[:, :ksz],
                                 sinvb[:, :ksz])

        # ---- conv0 weights (loaded during attention; fp8) ----
        w0x_s = persist.tile([128, 18, C], F8, tag="w0x")
        nc.sync.dma_start(
            out=w0x_s, in_=w0x_d.rearrange("t (a p) o -> p (t a) o", a=2))
        w0inh_s = persist.tile([128, 18, C], F8, tag="w0inh")
        nc.sync.dma_start(
            out=w0inh_s, in_=w0inh_d.rearrange("t (a p) o -> p (t a) o", a=2))
        w0inl_s = persist.tile([128, 18, C], F8, tag="w0inl")
        nc.sync.dma_start(
            out=w0inl_s, in_=w0inl_d.rearrange("t (a p) o -> p (t a) o", a=2))

        # ---- out-projection + bn0 + gelu -> x0' (fp8) into ybuf ----
        for kt, (k0, ksz) in enumerate(K_TILES):
            nr = ksz // W  # rows in this k-tile
            for oc in range(2):
                po = psum.tile([128, 2, 512], F32, tag="ps_S")
                nc.tensor.matmul(po[:, 0, :ksz],
                                 wo_s[:, oc * 128:(oc + 1) * 128],
                                 x0n_s[:, k0:k0 + ksz],
                                 start=True, stop=True)
                nc.scalar.activation(
                    ybuf[:, oc, 1 + kt * 8:1 + kt * 8 + nr, 1:65],
                    po[:, 0, :ksz].rearrange("p (r w) -> p r w", w=W),
                    AF.Gelu, bias=bo_s[:, oc:oc + 1], scale=ao_s[:, oc:oc + 1])

        # ---- conv1 weights (loaded during conv0; fp8 hi/lo) ----
        w1h_s = persist.tile([128, 18, C], F8, tag="w1h")
        nc.sync.dma_start(
            out=w1h_s, in_=w1h_d.rearrange("t (a p) o -> p (t a) o", a=2))
        w1l_s = persist.tile([128, 18, C], F8, tag="w1l")
        nc.sync.dma_start(
            out=w1l_s, in_=w1l_d.rearrange("t (a p) o -> p (t a) o", a=2))

        # ---- conv0: 512 -> 256, 3x3, all DoubleRow fp8, bn + gelu ----
        # x0'-half naive fp8 (9 passes) + in0-half 3-term hi/lo (27 passes)
        for r0, nr in ROW_TILES:
            for oc in range(2):
                pc = psum.tile([128, 2, 512], F32, tag="ps_S")
                pcv = pc[:, 0, :nr * W].rearrange("p (r w) -> p r w", w=W)
                terms0 = [(w0inh_s, in0h_s), (w0inh_s, in0l_s),
                          (w0inl_s, in0h_s), (w0x_s, ybuf)]
                i_mm, n_mm = 0, 9 * len(terms0)
                for w_s, x_s in terms0:   # in0 terms first: ready earliest
                    for t9 in range(9):
                        dh, dw = divmod(t9, 3)
                        nc.tensor.matmul(
                            pcv,
                            w_s[:, t9 * 2:t9 * 2 + 2, oc * 128:(oc + 1) * 128],
                            x_s[:, :, r0 + dh - 1:r0 + dh - 1 + nr, dw:dw + W],
                            start=(i_mm == 0), stop=(i_mm == n_mm - 1),
                            perf_mode=DR)
                        i_mm += 1
                nc.scalar.activation(
                    c0f[:, oc, r0 - 1:r0 - 1 + nr, :], pcv,
                    AF.Gelu, bias=b0_s[:, oc:oc + 1], scale=a0_s[:, oc:oc + 1])
                # hi/lo split of c0 for conv1's 3-term product (DVE)
                nc.vector.tensor_copy(c0h[:, oc, r0:r0 + nr, 1:65],
                                      c0f[:, oc, r0 - 1:r0 - 1 + nr, :])
                nc.vector.tensor_sub(c0l[:, oc, r0:r0 + nr, 1:65],
                                     c0f[:, oc, r0 - 1:r0 - 1 + nr, :],
                                     c0h[:, oc, r0:r0 + nr, 1:65])

        # ---- conv1: 256 -> 256, 3-term DoubleRow fp8, bn + gelu,
        #      + x0' residual, row-max ----
        outs = [persist.tile([128, ROWS], F32, tag=f"out{oc}", name=f"outs{oc}")
                for oc in range(2)]
        for r0, nr in ROW_TILES:
            for oc in range(2):
                pc = psum.tile([128, 2, 512], F32, tag="ps_S")
                pcv = pc[:, 0, :nr * W].rearrange("p (r w) -> p r w", w=W)
                terms1 = [(w1h_s, c0h), (w1h_s, c0l), (w1l_s, c0h)]
                i_mm, n_mm = 0, 9 * len(terms1)
                for w_s, x_s in terms1:
                    for t9 in range(9):
                        dh, dw = divmod(t9, 3)
                        nc.tensor.matmul(
                            pcv,
                            w_s[:, t9 * 2:t9 * 2 + 2, oc * 128:(oc + 1) * 128],
                            x_s[:, :, r0 + dh - 1:r0 + dh - 1 + nr, dw:dw + W],
                            start=(i_mm == 0), stop=(i_mm == n_mm - 1),
                            perf_mode=DR)
                        i_mm += 1
                tmp = small.tile([128, 512], F32, tag="scratch")
                nc.scalar.activation(tmp[:, :nr * W], pc[:, 0, :nr * W], AF.Gelu,
                                     bias=b1_s[:, oc:oc + 1],
                                     scale=a1_s[:, oc:oc + 1])
                res = small.tile([128, 512], F32, tag="scratch")
                nc.vector.tensor_add(
                    res[:, :nr * W].rearrange("p (r w) -> p r w", w=W),
                    tmp[:, :nr * W].rearrange("p (r w) -> p r w", w=W),
                    ybuf[:, oc, r0:r0 + nr, 1:65])
                nc.vector.reduce_max(
                    outs[oc][:, r0 - 1:r0 - 1 + nr],
                    res[:, :nr * W].rearrange("p (r w) -> p r w", w=W),
                    axis=AX.X)
        for oc in range(2):
            nc.sync.dma_start(out=out[oc * 128:(oc + 1) * 128, :], in_=outs[oc])

    nc.compile()
    return nc


def _prep_maps(inputs):
    """Host-side input prep: slicing, transposes, BN folding, fp8 splits."""
    f = np.float32
    in0 = np.ascontiguousarray(np.asarray(inputs["inputs_0"], f).reshape(B, C, N))
    in1 = np.ascontiguousarray(np.asarray(inputs["inputs_1"], f).reshape(B, C, N))
    g = {k: np.asarray(v, f) for k, v in inputs.items()}

    def fold(gm, bt, m, v, conv_b):
        a = (gm / np.sqrt(v + EPS)).astype(f)
        return a, (bt - m * a + a * conv_b).astype(f)

    a_bn, b_bn = fold(g["bn0_g"], g["bn0_b"], g["bn0_m"], g["bn0_v"],
                      g["b_o"] + g["w_o"] @ g["b_q"])
    a0, b0 = fold(g["cb_bn0_g"], g["cb_bn0_b"], g["cb_bn0_m"], g["cb_bn0_v"],
                  g["cb_b0"])
    a1, b1 = fold(g["cb_bn1_g"], g["cb_bn1_b"], g["cb_bn1_m"], g["cb_bn1_v"],
                  g["cb_b1"])

    def wsplit(w):
        wh = w.astype(F8NP)
        wl = (w - wh.astype(f)).astype(F8NP)
        return wh, wl

    # conv weights as (tap, ci, o); x0-half naive fp8, in0-half + w1 hi/lo
    w0t = np.ascontiguousarray(
        g["cb_w0"].transpose(2, 3, 1, 0).reshape(9, 2 * C, C))
    w1t = np.ascontiguousarray(
        g["cb_w1"].transpose(2, 3, 1, 0).reshape(9, C, C))
    w0inh, w0inl = wsplit(w0t[:, C:, :])
    w1h, w1l = wsplit(w1t)

    shared = {
        "wq_t": np.ascontiguousarray(g["w_q"].T).astype(BF16NP),
        "wk_t": np.ascontiguousarray(g["w_k"].T).astype(BF16NP),
        "wv_t": np.ascontiguousarray(g["w_v"].T).astype(BF16NP),
        "wo_t": np.ascontiguousarray(g["w_o"].T).astype(BF16NP),
        "bv": np.ascontiguousarray(g["b_v"].reshape(Ch, 1)),
        "ao": a_bn.reshape(C, 1), "bo": b_bn.reshape(C, 1),
        "w0x": np.ascontiguousarray(w0t[:, :C, :]).astype(F8NP),
        "w0inh": np.ascontiguousarray(w0inh),
        "w0inl": np.ascontiguousarray(w0inl),
        "a0": a0.reshape(C, 1), "b0": b0.reshape(C, 1),
        "w1h": np.ascontiguousarray(w1h),
        "w1l": np.ascontiguousarray(w1l),
        "a1": a1.reshape(C, 1), "b1": b1.reshape(C, 1),
    }
    maps = []
    for b in range(B):
        in0b16 = in0[b].astype(BF16NP)
        for half in range(2):
            w0r = 0 if half == 0 else 30
            sl = slice(w0r * W, (w0r + ROWS) * W)
            in0w_f32 = in0[b][:, sl].reshape(C, ROWS, W)
            in0h = np.zeros((C, ROWS, 66), F8NP)
            in0l = np.zeros((C, ROWS, 66), F8NP)
            in0h[:, :, 1:65] = in0w_f32.astype(F8NP)
            in0l[:, :, 1:65] = (
                in0w_f32 - in0h[:, :, 1:65].astype(f)).astype(F8NP)
            maps.append({
                "in0b": in0b16,
                "in0h": in0h.reshape(C, ROWS * 66),
                "in0l": in0l.reshape(C, ROWS * 66),
                "in1b": np.ascontiguousarray(in1[b][:, sl]).astype(BF16NP),
                **shared,
            })
    return maps


def kernel(**inputs):
    if "nc" not in _CACHED:
        _CACHED["nc"] = build_program()
    nc = _CACHED["nc"]
    maps = _prep_maps(inputs)
    res = run_bass_kernel_spmd(nc, maps, core_ids=list(range(8)))
    out = np.zeros((B, C), np.float32)
    for b in range(B):
        top = res.results[2 * b]["out"][:, 0:32].max(axis=1)
        bot = res.results[2 * b + 1]["out"][:, 2:34].max(axis=1)
        out[b] = np.maximum(out[b], np.maximum(top, bot))
    return out
